# revision 10
# baseline (speedup 1.0000x reference)
"""CrissCrossAttention Trainium2 kernel (8 NeuronCores, data-parallel).

Problem: B=4, C=256, H=W=128, 4 heads. Per head: cq=8 q/k channels, cv=64
v channels. Row attention (over W per row) + column attention (over H per
column), outputs added with the CCNet spatial-transpose quirk, then
out = gamma*attn + x.

Sharding: 16 (batch, head) pairs over 8 cores -> each core handles
batch b = core//2 and head pair p = core%2 (global heads 2p, 2p+1).
Each core reads x[b] (all 256 channels, needed by the projections) and
produces output channels [128p : 128p+128] of batch b.

Host-side prep per core: x is bf16, channel-reordered so the residual
slice is rows 0-127 of x_in; weight rows permuted identically. A second
pixel-major copy x_pix[w, h*128+ch] feeds the residual add. Output is
bf16 pixel-major [w, h*128+ch]; host transposes/upcasts.

Structure (pixels indexed pix = h*128 + w):
  Phase A (fused projection + ROW attention, one 4-row group per iter):
    - x chunk load (bf16), qk projection -> PSUM, evacuated by ACT
      (Copy + per-partition bias) into a rotating fr[32, 512] tile.
    - vT projection (pixel-major vT[128w, 128h, 130c] bf16, channels
      [64 h0 | 1 | 64 h1 | 1], ones for the softmax denominator); DVE
      evacuation adds the replicated bias.
    - fc[32, w*128+h] col-major flat store built incrementally (DVE).
    - band-pack q/k rows of the group (SBUF->SBUF DMA): q/k of row h at
      partition 32*(h%4)+c so the 4 rows hit distinct PE row-groups.
    - row attention for the PREVIOUS group, software-pipelined: 8 energy
      matmuls (K=8, 4-way concurrent via tile_position into pe's 4
      banks), one EXP (ACT) covering both heads, 8 value matmuls po
      (65th column = denominator), reciprocal + multiply (DVE) into a
      persistent til_all stash.
  Phase B2: qc/kc band-pack from fc (DMA) + vTc[h, w, c] spatial
    transpose of vT via per-channel PE transposes.
  Phase C (COLUMN attention, pipelined one group deep): energies/EXP/po
    as above using qc/kc/vTc; CCNet combine t_row (stash) + t_col
    (GpSimd); residual add in pixel-major (DVE); DMA out bf16.
"""

import os
import numpy as np
from contextlib import ExitStack

import concourse.bass as bass
import concourse.bacc as bacc
import concourse.tile as tile
from concourse import mybir
from concourse.masks import make_identity

F32 = mybir.dt.float32
BF16 = mybir.dt.bfloat16

B, C, H, W = 4, 256, 128, 128
PIX = H * W            # 16384
CV = 64                # v channels per head
NCORES = 8
G = 4                  # rows per attention group (= PE row-group packing)
NG = H // G            # 32 groups


def build_program():
    nc = bacc.Bacc("TRN2", target_bir_lowering=False, debug=False,
                   num_devices=NCORES)

    x_in = nc.dram_tensor("x_in", [C, PIX], BF16, kind="ExternalInput")
    x_pix = nc.dram_tensor("x_pix", [W, H * 128], BF16, kind="ExternalInput")
    wqkT = nc.dram_tensor("wqkT", [C, 32], BF16, kind="ExternalInput")
    qk_bias = nc.dram_tensor("qk_bias", [32, 1], F32, kind="ExternalInput")
    wvT = nc.dram_tensor("wvT", [C, 130], BF16, kind="ExternalInput")
    vbias_full = nc.dram_tensor("vbias_full", [128, 130], BF16,
                                kind="ExternalInput")
    out = nc.dram_tensor("out", [W, H * 128], BF16, kind="ExternalOutput")

    with tile.TileContext(nc) as tc, ExitStack() as ctx:
        consts = ctx.enter_context(tc.tile_pool(name="consts", bufs=1))
        persist = ctx.enter_context(tc.tile_pool(name="persist", bufs=1))

        # constants / weights
        wqa = consts.tile([128, 32], BF16, tag="wqa")
        wqb = consts.tile([128, 32], BF16, tag="wqb")
        nc.sync.dma_start(wqa, wqkT[0:128, :])
        nc.sync.dma_start(wqb, wqkT[128:256, :])
        wva = consts.tile([128, 130], BF16, tag="wva")
        wvb = consts.tile([128, 130], BF16, tag="wvb")
        nc.sync.dma_start(wva, wvT[0:128, :])
        nc.sync.dma_start(wvb, wvT[128:256, :])
        qkb = consts.tile([32, 1], F32, tag="qkb")
        nc.sync.dma_start(qkb, qk_bias[:, :])
        vbias = consts.tile([128, 1, 130], BF16, tag="vbias")
        nc.sync.dma_start(vbias[:, 0, :], vbias_full[:, :])
        identb = consts.tile([128, 128], BF16, tag="identb")
        make_identity(nc, identb)

        # persistent activations
        q_sb = persist.tile([128, 2, NG, W], BF16, tag="q")    # 16 KiB
        k_sb = persist.tile([128, 2, NG, W], BF16, tag="k")    # 16 KiB
        qc_sb = persist.tile([128, 2, NG, H], BF16, tag="qc")  # 16 KiB
        kc_sb = persist.tile([128, 2, NG, H], BF16, tag="kc")  # 16 KiB
        vT_sb = persist.tile([128, H, 130], BF16, tag="vT")    # 32.5 KiB
        # row-attention results stash: [w, group, row-in-group, head, cv]
        til_all = persist.tile([128, NG, G, 2, CV], BF16, tag="tA")  # 32 KiB

        CHUNK = 512   # pixels per iteration = one 4-row group
        NCH = PIX // CHUNK   # 32

        # ---------- Phase A: projections + row attention ----------
        with (
            tc.tile_pool(name="fcpool", bufs=1) as fcpool,
            tc.tile_pool(name="frpool", bufs=2) as frpool,
            tc.tile_pool(name="xchunk", bufs=2) as xpool,
            tc.tile_pool(name="ptrow", bufs=2) as ptpoolr,
            tc.tile_pool(name="rcrow", bufs=2) as rcpoolr,
            tc.tile_pool(name="pq", bufs=1, space="PSUM") as pqpool,
            tc.tile_pool(name="pv", bufs=1, space="PSUM") as pvpool,
            tc.tile_pool(name="per", bufs=1, space="PSUM") as pepoolr,
            tc.tile_pool(name="por", bufs=1, space="PSUM") as popoolr,
        ):
            fc = fcpool.tile([32, PIX], BF16, tag="fc")  # [c, w*128+h]
            fcv = fc[:, :].rearrange("c (w h) -> c w h", h=H)

            frs = [None] * NCH

            def proj(chi):
                c0 = chi * CHUNK
                r0 = chi * G
                eng = nc.sync if chi % 2 == 0 else nc.scalar
                xab = xpool.tile([128, CHUNK], BF16, tag="xab")
                xbb = xpool.tile([128, CHUNK], BF16, tag="xbb")
                eng.dma_start(xab, x_in[0:128, c0 : c0 + CHUNK])
                eng.dma_start(xbb, x_in[128:256, c0 : c0 + CHUNK])
                xav = xab[:, :].rearrange("p (r w) -> p r w", w=128)
                xbv = xbb[:, :].rearrange("p (r w) -> p r w", w=128)

                # qk projection; ACT evacuation with per-partition bias
                fr = frpool.tile([32, CHUNK], BF16, tag="fr")
                frs[chi] = fr
                pq = pqpool.tile([32, 512], F32, tag="pq")
                nc.tensor.matmul(pq, wqa, xab[:, :], start=True, stop=False)
                nc.tensor.matmul(pq, wqb, xbb[:, :], start=False, stop=True)
                nc.scalar.activation(fr, pq,
                                     mybir.ActivationFunctionType.Identity,
                                     bias=qkb)

                # vT projection: two 2-row PSUM tiles; DVE bias evacuation
                for s2 in range(2):
                    pv = pvpool.tile([128, 2, 130], F32, tag="pv")
                    for s3 in range(2):
                        srow = 2 * s2 + s3
                        nc.tensor.matmul(pv[:, s3, :], xav[:, srow, :], wva,
                                         start=(s3 == 0), stop=False,
                                         skip_group_check=True)
                        nc.tensor.matmul(pv[:, s3, :], xbv[:, srow, :], wvb,
                                         start=False, stop=(s3 == 1),
                                         skip_group_check=True)
                    nc.vector.tensor_tensor(
                        vT_sb[:, r0 + 2 * s2 : r0 + 2 * s2 + 2, :], pv,
                        vbias.to_broadcast((128, 2, 130)),
                        mybir.AluOpType.add)

                # col-major flat store slice (needs only this chunk's rows)
                frv = fr[:, :].rearrange("c (h w) -> c w h", w=128)
                nc.vector.tensor_copy(fcv[:, :, r0 : r0 + G], frv)

                # band-pack this group's q/k rows
                srcv = fr[:, :].rearrange("c (b w) -> c b w", w=128)
                for bb in range(4):
                    for hh in range(2):
                        eng = nc.sync if (bb + hh) % 2 == 0 else nc.scalar
                        eng.dma_start(
                            q_sb[32 * bb : 32 * bb + 8, hh, chi, :],
                            srcv[8 * hh : 8 * hh + 8, bb, :])
                        eng.dma_start(
                            k_sb[32 * bb : 32 * bb + 8, hh, chi, :],
                            srcv[16 + 8 * hh : 24 + 8 * hh, bb, :])

            def energies(g, qs, ks, pepool, ptpool):
                pe = pepool.tile([128, G, 512], F32, tag="pe")
                for hh in range(2):
                    for j in range(G):
                        nc.tensor.matmul(
                            pe[:, j, 128 * hh : 128 * hh + 128],
                            ks[32 * j : 32 * j + 8, hh, g, :],
                            qs[32 * j : 32 * j + 8, hh, g, :],
                            start=True, stop=True,
                            tile_position=(32 * j, 0),
                            skip_group_check=True,
                        )
                pT = ptpool.tile([128, G, 256], BF16, tag="pt")
                nc.scalar.activation(pT, pe[:, :, 0:256],
                                     mybir.ActivationFunctionType.Exp)
                return pT

            def po_matmuls(g, pT, vs, popool):
                po = popool.tile([128, 2, G, 128], F32, tag="po")
                for hh in range(2):
                    for j in range(G):
                        nc.tensor.matmul(
                            po[:, hh, j, 0:65],
                            pT[:, j, 128 * hh : 128 * hh + 128],
                            vs[:, g * G + j, 65 * hh : 65 * hh + 65],
                            start=True, stop=True,
                            skip_group_check=True,
                        )
                return po

            def rowdiv(g, po, rcpool):
                # divide by denominator; write into the persistent stash
                pov = po[:, :, :, :].rearrange("p hh j c -> p j hh c")
                rec = rcpool.tile([128, G, 2, 1], F32, tag="rc")
                nc.vector.reciprocal(rec, pov[:, :, :, 64:65])
                nc.vector.tensor_tensor(
                    til_all[:, g, :, :, :], pov[:, :, :, 0:64],
                    rec.to_broadcast((128, G, 2, CV)),
                    mybir.AluOpType.mult,
                )

            proj(0)
            for it in range(1, NCH + 1):
                gp = it - 1
                pT = energies(gp, q_sb, k_sb, pepoolr, ptpoolr)
                if it < NCH:
                    proj(it)
                po = po_matmuls(gp, pT, vT_sb, popoolr)
                rowdiv(gp, po, rcpoolr)

            # column-direction band-pack (needs the full fc)
            src_c = fc[:, :].rearrange("c (wb b h) -> c b wb h", b=4, h=H)
            ei = 0
            engs = [nc.sync, nc.scalar, nc.gpsimd]
            for bb in range(4):
                for hh in range(2):
                    eng = engs[ei % 3]
                    ei += 1
                    eng.dma_start(
                        qc_sb[32 * bb : 32 * bb + 8, hh, :, :],
                        src_c[8 * hh : 8 * hh + 8, bb, :, :])
                    eng.dma_start(
                        kc_sb[32 * bb : 32 * bb + 8, hh, :, :],
                        src_c[16 + 8 * hh : 24 + 8 * hh, bb, :, :])

        # ---------- Phase B2 + C: vTc transpose, column attention ----------
        with (
            tc.tile_pool(name="vtc", bufs=1) as vtcpool,
            tc.tile_pool(name="ptcol", bufs=4) as ptpoolc,
            tc.tile_pool(name="ttc", bufs=3) as tpoolc,
            tc.tile_pool(name="rccol", bufs=4) as rcpoolc,
            tc.tile_pool(name="au", bufs=2) as aupool,
            tc.tile_pool(name="io", bufs=3) as iopool,
        ):
            vTc_sb = vtcpool.tile([128, W, 130], BF16, tag="vTc")  # 32.5 KiB

            with tc.tile_pool(name="ptr", bufs=2, space="PSUM") as ptrpool:
                for cb in range(33):
                    nch = min(4, 130 - cb * 4)
                    ptr = ptrpool.tile([128, 4, 128], BF16, tag="ptr")
                    for cj in range(nch):
                        cch = cb * 4 + cj
                        nc.tensor.matmul(ptr[:, cj, :], vT_sb[:, :, cch],
                                         identb, start=True, stop=True,
                                         is_transpose=True)
                    nc.vector.tensor_copy(
                        vTc_sb[:, :, cb * 4 : cb * 4 + nch],
                        ptr[:, 0:nch, :].rearrange("p c w -> p w c"))

            with (
                tc.tile_pool(name="pec", bufs=1, space="PSUM") as pepoolc,
                tc.tile_pool(name="poc", bufs=2, space="PSUM") as popoolc,
            ):
                def energies(g):
                    pe = pepoolc.tile([128, G, 512], F32, tag="pe")
                    for hh in range(2):
                        for j in range(G):
                            nc.tensor.matmul(
                                pe[:, j, 128 * hh : 128 * hh + 128],
                                kc_sb[32 * j : 32 * j + 8, hh, g, :],
                                qc_sb[32 * j : 32 * j + 8, hh, g, :],
                                start=True, stop=True,
                                tile_position=(32 * j, 0),
                                skip_group_check=True,
                            )
                    pT = ptpoolc.tile([128, G, 256], BF16, tag="pt")
                    nc.scalar.activation(pT, pe[:, :, 0:256],
                                         mybir.ActivationFunctionType.Exp)
                    return pT

                def attend(g, pT):
                    po = popoolc.tile([128, 2, G, 128], F32, tag="po")
                    for hh in range(2):
                        for j in range(G):
                            nc.tensor.matmul(
                                po[:, hh, j, 0:65],
                                pT[:, j, 128 * hh : 128 * hh + 128],
                                vTc_sb[:, g * G + j, 65 * hh : 65 * hh + 65],
                                start=True, stop=True,
                                skip_group_check=True,
                            )
                    til = tpoolc.tile([128, G, 2, CV], BF16, tag="t")
                    pov = po[:, :, :, :].rearrange("p hh j c -> p j hh c")
                    rec = rcpoolc.tile([128, G, 2, 1], F32, tag="rc")
                    nc.vector.reciprocal(rec, pov[:, :, :, 64:65])
                    nc.vector.tensor_tensor(
                        til, pov[:, :, :, 0:64],
                        rec.to_broadcast((128, G, 2, CV)),
                        mybir.AluOpType.mult,
                    )
                    au = aupool.tile([128, G, 128], BF16, tag="au")
                    nc.gpsimd.tensor_tensor(
                        au, til_all[:, g, :, :, :], til[:, :, :, :],
                        mybir.AluOpType.add)
                    eng = nc.sync if g % 2 == 0 else nc.scalar
                    xpg = iopool.tile([128, G * 128], BF16, tag="xpg")
                    eng.dma_start(xpg, x_pix[:, g * 512 : (g + 1) * 512])
                    res = iopool.tile([128, G * 128], BF16, tag="res")
                    nc.vector.tensor_tensor(
                        res, au[:, :, :].rearrange("p g w -> p (g w)"),
                        xpg, mybir.AluOpType.add)
                    eng.dma_start(out[:, g * 512 : (g + 1) * 512], res)

                prev = energies(0)
                for g in range(1, NG):
                    cur = energies(g)
                    attend(g - 1, prev)
                    prev = cur
                attend(NG - 1, prev)

    return nc


def _prep_core_inputs(core, x, Wq, bq, Wk, bk, Wv, bv, gamma):
    b = core // 2
    p = core % 2
    g = float(np.asarray(gamma).reshape(-1)[0])
    qsl = slice(16 * p, 16 * p + 16)
    vsl = slice(128 * p, 128 * p + 128)

    import ml_dtypes
    bf = ml_dtypes.bfloat16

    # channel permutation: residual (output) channels first
    perm = np.concatenate([np.arange(128 * p, 128 * p + 128),
                           np.arange(128 * (1 - p), 128 * (1 - p) + 128)])

    wqk = np.zeros((C, 32), np.float32)
    wqk[:, 0:16] = Wq[qsl].T       # q head even(8) | q head odd(8)
    wqk[:, 16:32] = Wk[qsl].T
    wqk = wqk[perm].astype(bf)
    qkb = np.concatenate([bq[qsl], bk[qsl]]).reshape(32, 1).astype(np.float32)

    wv_eff = (g * Wv[vsl]).astype(np.float32)     # [128, 256]
    bv_eff = (g * bv[vsl]).astype(np.float32)
    wvt = np.zeros((C, 130), np.float32)
    wvt[:, 0:64] = wv_eff[0:64].T
    wvt[:, 65:129] = wv_eff[64:128].T
    wvt = wvt[perm].astype(bf)
    vbias = np.zeros((1, 130), np.float32)
    vbias[0, 0:64] = bv_eff[0:64]
    vbias[0, 64] = 1.0
    vbias[0, 65:129] = bv_eff[64:128]
    vbias[0, 129] = 1.0
    vbias_full = np.broadcast_to(vbias, (128, 130)).astype(bf)

    x2 = x[b].reshape(C, PIX)[perm]
    # pixel-major residual: [w][h][ch] so per-group loads are contiguous
    xpix = np.ascontiguousarray(
        x[b, vsl].transpose(2, 1, 0)).reshape(W, H * 128)
    return {
        "x_in": np.ascontiguousarray(x2).astype(bf),
        "x_pix": xpix.astype(bf),
        "wqkT": wqk,
        "qk_bias": qkb,
        "wvT": wvt,
        "vbias_full": np.ascontiguousarray(vbias_full),
    }


_NC_CACHE = None


def _get_nc():
    global _NC_CACHE
    if _NC_CACHE is None:
        nc = build_program()
        nc.compile()
        _NC_CACHE = nc
    return _NC_CACHE


def kernel(x, Wq, bq, Wk, bk, Wv, bv, gamma, _trace=False, _trace_kwargs=None):
    from concourse.bass_utils import run_bass_kernel_spmd

    nc = _get_nc()
    in_maps = [
        _prep_core_inputs(core, x, Wq, bq, Wk, bk, Wv, bv, gamma)
        for core in range(NCORES)
    ]
    res = run_bass_kernel_spmd(
        nc, in_maps, list(range(NCORES)), trace=_trace,
        **(_trace_kwargs or {}),
    )
    outp = np.empty((B, C, H, W), np.float32)
    for core in range(NCORES):
        b, p = core // 2, core % 2
        o = res.results[core]["out"].astype(np.float32)
        # out[w, h*128+ch] -> [ch, h, w]
        outp[b, 128 * p : 128 * p + 128] = (
            o.reshape(W, H, 128).transpose(2, 1, 0)
        )
    if _trace:
        kernel.last_results = res
    return outp


# revision 12
# speedup vs baseline: 1.1079x; 1.1079x over previous
"""CrissCrossAttention Trainium2 kernel (8 NeuronCores, data-parallel).

Problem: B=4, C=256, H=W=128, 4 heads. Per head: cq=8 q/k channels, cv=64
v channels. Row attention (over W per row) + column attention (over H per
column), outputs added with the CCNet spatial-transpose quirk, then
out = gamma*attn + x.

Sharding: 16 (batch, head) pairs over 8 cores -> core handles batch
b = core//2, head pair p = core%2. Host prep: x bf16, channel-reordered
(residual channels first); pixel-major x_pix[w, h*128+ch] for the
residual; output bf16 pixel-major, transposed/upcast on host.

Phase A (fused projection + ROW attention, one 4-row group per iter):
  - qk projection -> PSUM, ACT evacuation (Identity + per-partition
    bias) into a rotating 8-chunk batch store frB[32, 8*512].
  - vT projection (pixel-major vT[128w, 128h, 130c], ones channels for
    the softmax denominator); DVE evacuation adds replicated bias.
  - fc[32, w*128+h] col-major flat store built incrementally (DVE).
  - every 8 chunks: batched q/k band-pack DMAs (16 dma_starts/batch;
    dma_start costs ~660ns of sequencer time, so batching matters).
    Band layout: q/k of row h at partition 32*(h%4)+c so a group's 4
    rows hit distinct PE row-groups -> 4-way concurrent energy matmuls.
  - row attention runs LAG=9 groups behind the projection: 8 energy
    matmuls (K=8, tile_position banks), one EXP (ACT) for both heads,
    8 value matmuls po (65th col = denominator), reciprocal + multiply
    (DVE) into the persistent til_all stash.
Phase B2: qc/kc band-pack from fc + vTc via per-channel PE transposes.
Phase C (COLUMN attention, pipelined one group deep): energies/EXP/po
  with qc/kc/vTc; combine stash+col (GpSimd); residual add (DVE); out.
"""

import os
import numpy as np
from contextlib import ExitStack

import concourse.bass as bass
import concourse.bacc as bacc
import concourse.tile as tile
from concourse import mybir
from concourse.masks import make_identity

F32 = mybir.dt.float32
BF16 = mybir.dt.bfloat16

B, C, H, W = 4, 256, 128, 128
PIX = H * W            # 16384
CV = 64                # v channels per head
NCORES = 8
G = 4                  # rows per attention group (= PE row-group packing)
NG = H // G            # 32 groups
LAG = 9                # row attention trails projection by LAG groups


def build_program():
    nc = bacc.Bacc("TRN2", target_bir_lowering=False, debug=False,
                   num_devices=NCORES)

    x_in = nc.dram_tensor("x_in", [C, PIX], BF16, kind="ExternalInput")
    x_pix = nc.dram_tensor("x_pix", [W, H * 128], BF16, kind="ExternalInput")
    wqkT = nc.dram_tensor("wqkT", [C, 32], BF16, kind="ExternalInput")
    qk_bias = nc.dram_tensor("qk_bias", [32, 1], F32, kind="ExternalInput")
    wvT = nc.dram_tensor("wvT", [C, 130], BF16, kind="ExternalInput")
    vbias_full = nc.dram_tensor("vbias_full", [128, 130], BF16,
                                kind="ExternalInput")
    out = nc.dram_tensor("out", [W, H * 128], BF16, kind="ExternalOutput")

    with tile.TileContext(nc) as tc, ExitStack() as ctx:
        consts = ctx.enter_context(tc.tile_pool(name="consts", bufs=1))
        persist = ctx.enter_context(tc.tile_pool(name="persist", bufs=1))

        wqa = consts.tile([128, 32], BF16, tag="wqa")
        wqb = consts.tile([128, 32], BF16, tag="wqb")
        nc.sync.dma_start(wqa, wqkT[0:128, :])
        nc.sync.dma_start(wqb, wqkT[128:256, :])
        wva = consts.tile([128, 130], BF16, tag="wva")
        wvb = consts.tile([128, 130], BF16, tag="wvb")
        nc.sync.dma_start(wva, wvT[0:128, :])
        nc.sync.dma_start(wvb, wvT[128:256, :])
        qkb = consts.tile([32, 1], F32, tag="qkb")
        nc.sync.dma_start(qkb, qk_bias[:, :])
        vbias = consts.tile([128, 1, 130], BF16, tag="vbias")
        nc.sync.dma_start(vbias[:, 0, :], vbias_full[:, :])
        identb = consts.tile([128, 128], BF16, tag="identb")
        make_identity(nc, identb)

        # persistent activations
        q_sb = persist.tile([128, 2, NG, W], BF16, tag="q")    # 16 KiB
        k_sb = persist.tile([128, 2, NG, W], BF16, tag="k")    # 16 KiB
        qc_sb = persist.tile([128, 2, NG, H], BF16, tag="qc")  # 16 KiB
        kc_sb = persist.tile([128, 2, NG, H], BF16, tag="kc")  # 16 KiB
        vT_sb = persist.tile([128, H, 130], BF16, tag="vT")    # 32.5 KiB
        til_all = persist.tile([128, NG, G, 2, CV], BF16, tag="tA")  # 32 KiB

        CHUNK = 512          # one 4-row group per chunk
        NCH = PIX // CHUNK   # 32
        BATCH = 8            # chunks per band-pack batch

        with (
            tc.tile_pool(name="fcpool", bufs=1) as fcpool,
            tc.tile_pool(name="frpool", bufs=2) as frpool,
            tc.tile_pool(name="xchunk", bufs=2) as xpool,
            tc.tile_pool(name="ptrow", bufs=2) as ptpoolr,
            tc.tile_pool(name="rcrow", bufs=2) as rcpoolr,
            tc.tile_pool(name="pq", bufs=1, space="PSUM") as pqpool,
            tc.tile_pool(name="pv", bufs=1, space="PSUM") as pvpool,
            tc.tile_pool(name="per", bufs=1, space="PSUM") as pepoolr,
            tc.tile_pool(name="por", bufs=1, space="PSUM") as popoolr,
        ):
            fc = fcpool.tile([32, PIX], BF16, tag="fc")  # [c, w*128+h]
            fcv = fc[:, :].rearrange("c (w h) -> c w h", h=H)

            state = {"frB": None}

            def proj(chi):
                c0 = chi * CHUNK
                r0 = chi * G
                ci = chi % BATCH
                eng = nc.sync if chi % 2 == 0 else nc.scalar
                xab = xpool.tile([128, CHUNK], BF16, tag="xab")
                xbb = xpool.tile([128, CHUNK], BF16, tag="xbb")
                eng.dma_start(xab, x_in[0:128, c0 : c0 + CHUNK])
                eng.dma_start(xbb, x_in[128:256, c0 : c0 + CHUNK])
                xav = xab[:, :].rearrange("p (r w) -> p r w", w=128)
                xbv = xbb[:, :].rearrange("p (r w) -> p r w", w=128)

                if ci == 0:
                    state["frB"] = frpool.tile([32, BATCH * CHUNK], BF16,
                                               tag="frB", name="frB")
                frB = state["frB"]

                # qk projection; ACT evacuation with per-partition bias
                pq = pqpool.tile([32, 512], F32, tag="pq")
                nc.tensor.matmul(pq, wqa, xab[:, :], start=True, stop=False)
                nc.tensor.matmul(pq, wqb, xbb[:, :], start=False, stop=True)
                nc.scalar.activation(frB[:, ci * CHUNK : (ci + 1) * CHUNK],
                                     pq,
                                     mybir.ActivationFunctionType.Identity,
                                     bias=qkb)

                # vT projection: two 2-row PSUM tiles; DVE bias evacuation.
                # pv bufs=1: the caller places po() between the two halves
                # so the PE covers the pv0-evacuation WAR wait.
                pvs = []
                for s2 in range(2):
                    pv = pvpool.tile([128, 2, 130], F32, tag="pv")
                    for s3 in range(2):
                        srow = 2 * s2 + s3
                        nc.tensor.matmul(pv[:, s3, :], xav[:, srow, :], wva,
                                         start=(s3 == 0), stop=False,
                                         skip_group_check=True)
                        nc.tensor.matmul(pv[:, s3, :], xbv[:, srow, :], wvb,
                                         start=False, stop=(s3 == 1),
                                         skip_group_check=True)
                    nc.vector.tensor_tensor(
                        vT_sb[:, r0 + 2 * s2 : r0 + 2 * s2 + 2, :], pv,
                        vbias.to_broadcast((128, 2, 130)),
                        mybir.AluOpType.add)
                    pvs.append(pv)

                # col-major flat store slice
                frv = frB[:, ci * CHUNK : (ci + 1) * CHUNK].rearrange(
                    "c (h w) -> c w h", w=128)
                nc.vector.tensor_copy(fcv[:, :, r0 : r0 + G], frv)

                if ci == BATCH - 1:
                    # batched band-pack of this batch's 8 groups
                    hb0 = (chi // BATCH) * BATCH
                    srcv = frB[:, :].rearrange("c (hb b w) -> c b hb w",
                                               b=4, w=128)
                    for bb in range(4):
                        for hh in range(2):
                            eng2 = nc.sync if (bb + hh) % 2 == 0 else nc.scalar
                            eng2.dma_start(
                                q_sb[32 * bb : 32 * bb + 8, hh,
                                     hb0 : hb0 + BATCH, :],
                                srcv[8 * hh : 8 * hh + 8, bb, :, :])
                            eng2.dma_start(
                                k_sb[32 * bb : 32 * bb + 8, hh,
                                     hb0 : hb0 + BATCH, :],
                                srcv[16 + 8 * hh : 24 + 8 * hh, bb, :, :])

            def energies(g, qs, ks, pepool, ptpool):
                pe = pepool.tile([128, G, 512], F32, tag="pe")
                for hh in range(2):
                    for j in range(G):
                        nc.tensor.matmul(
                            pe[:, j, 128 * hh : 128 * hh + 128],
                            ks[32 * j : 32 * j + 8, hh, g, :],
                            qs[32 * j : 32 * j + 8, hh, g, :],
                            start=True, stop=True,
                            tile_position=(32 * j, 0),
                            skip_group_check=True,
                        )
                pT = ptpool.tile([128, G, 256], BF16, tag="pt")
                nc.scalar.activation(pT, pe[:, :, 0:256],
                                     mybir.ActivationFunctionType.Exp)
                return pT

            def po_matmuls(g, pT, vs, popool):
                po = popool.tile([128, 2, G, 128], F32, tag="po")
                for hh in range(2):
                    for j in range(G):
                        nc.tensor.matmul(
                            po[:, hh, j, 0:65],
                            pT[:, j, 128 * hh : 128 * hh + 128],
                            vs[:, g * G + j, 65 * hh : 65 * hh + 65],
                            start=True, stop=True,
                            skip_group_check=True,
                        )
                return po

            def rowdiv(g, po, rcpool):
                pov = po[:, :, :, :].rearrange("p hh j c -> p j hh c")
                rec = rcpool.tile([128, G, 2, 1], F32, tag="rc")
                nc.vector.reciprocal(rec, pov[:, :, :, 64:65])
                nc.vector.tensor_tensor(
                    til_all[:, g, :, :, :], pov[:, :, :, 0:64],
                    rec.to_broadcast((128, G, 2, CV)),
                    mybir.AluOpType.mult,
                )

            for it in range(NCH + LAG):
                gp = it - LAG
                pT = None
                if gp >= 0:
                    pT = energies(gp, q_sb, k_sb, pepoolr, ptpoolr)
                if it < NCH:
                    proj(it)
                if pT is not None:
                    po = po_matmuls(gp, pT, vT_sb, popoolr)
                    rowdiv(gp, po, rcpoolr)

            # column-direction band-pack (needs the full fc)
            src_c = fc[:, :].rearrange("c (wb b h) -> c b wb h", b=4, h=H)
            ei = 0
            engs = [nc.sync, nc.scalar, nc.gpsimd]
            for bb in range(4):
                for hh in range(2):
                    eng = engs[ei % 3]
                    ei += 1
                    eng.dma_start(
                        qc_sb[32 * bb : 32 * bb + 8, hh, :, :],
                        src_c[8 * hh : 8 * hh + 8, bb, :, :])
                    eng.dma_start(
                        kc_sb[32 * bb : 32 * bb + 8, hh, :, :],
                        src_c[16 + 8 * hh : 24 + 8 * hh, bb, :, :])

        # ---------- Phase B2 + C: vTc transpose, column attention ----------
        with (
            tc.tile_pool(name="vtc", bufs=1) as vtcpool,
            tc.tile_pool(name="ptcol", bufs=4) as ptpoolc,
            tc.tile_pool(name="ttc", bufs=3) as tpoolc,
            tc.tile_pool(name="rccol", bufs=4) as rcpoolc,
            tc.tile_pool(name="au", bufs=2) as aupool,
            tc.tile_pool(name="io", bufs=2) as iopool,
        ):
            vTc_sb = vtcpool.tile([128, W, 130], BF16, tag="vTc")  # 32.5 KiB

            with tc.tile_pool(name="ptr", bufs=2, space="PSUM") as ptrpool:
                for cb in range(33):
                    nch = min(4, 130 - cb * 4)
                    ptr = ptrpool.tile([128, 4, 128], BF16, tag="ptr")
                    for cj in range(nch):
                        cch = cb * 4 + cj
                        nc.tensor.matmul(ptr[:, cj, :], vT_sb[:, :, cch],
                                         identb, start=True, stop=True,
                                         is_transpose=True)
                    nc.vector.tensor_copy(
                        vTc_sb[:, :, cb * 4 : cb * 4 + nch],
                        ptr[:, 0:nch, :].rearrange("p c w -> p w c"))

            with (
                tc.tile_pool(name="pec", bufs=1, space="PSUM") as pepoolc,
                tc.tile_pool(name="poc", bufs=2, space="PSUM") as popoolc,
            ):
                def energies_c(g):
                    pe = pepoolc.tile([128, G, 512], F32, tag="pe")
                    for hh in range(2):
                        for j in range(G):
                            nc.tensor.matmul(
                                pe[:, j, 128 * hh : 128 * hh + 128],
                                kc_sb[32 * j : 32 * j + 8, hh, g, :],
                                qc_sb[32 * j : 32 * j + 8, hh, g, :],
                                start=True, stop=True,
                                tile_position=(32 * j, 0),
                                skip_group_check=True,
                            )
                    pT = ptpoolc.tile([128, G, 256], BF16, tag="pt")
                    nc.scalar.activation(pT, pe[:, :, 0:256],
                                         mybir.ActivationFunctionType.Exp)
                    return pT

                state_c = {"xpg": None, "res": None}

                def attend_c(g, pT):
                    po = popoolc.tile([128, 2, G, 128], F32, tag="po")
                    for hh in range(2):
                        for j in range(G):
                            nc.tensor.matmul(
                                po[:, hh, j, 0:65],
                                pT[:, j, 128 * hh : 128 * hh + 128],
                                vTc_sb[:, g * G + j, 65 * hh : 65 * hh + 65],
                                start=True, stop=True,
                                skip_group_check=True,
                            )
                    til = tpoolc.tile([128, G, 2, CV], BF16, tag="t")
                    pov = po[:, :, :, :].rearrange("p hh j c -> p j hh c")
                    rec = rcpoolc.tile([128, G, 2, 1], F32, tag="rc")
                    nc.vector.reciprocal(rec, pov[:, :, :, 64:65])
                    nc.vector.tensor_tensor(
                        til, pov[:, :, :, 0:64],
                        rec.to_broadcast((128, G, 2, CV)),
                        mybir.AluOpType.mult,
                    )
                    au = aupool.tile([128, G, 128], BF16, tag="au")
                    nc.gpsimd.tensor_tensor(
                        au, til_all[:, g, :, :, :], til[:, :, :, :],
                        mybir.AluOpType.add)
                    # pair groups per xpg/out DMA to halve dma_start count
                    eng = nc.sync if g % 4 < 2 else nc.scalar
                    if g % 2 == 0:
                        state_c["xpg"] = iopool.tile([128, 2, G * 128], BF16,
                                                     tag="xpg", name="xpg")
                        state_c["res"] = iopool.tile([128, 2, G * 128], BF16,
                                                     tag="res", name="res")
                        eng.dma_start(state_c["xpg"],
                                      x_pix[:, g * 512 : (g + 2) * 512])
                    xpg, res2 = state_c["xpg"], state_c["res"]
                    nc.vector.tensor_tensor(
                        res2[:, g % 2, :],
                        au[:, :, :].rearrange("p g w -> p (g w)"),
                        xpg[:, g % 2, :], mybir.AluOpType.add)
                    if g % 2 == 1:
                        eng.dma_start(out[:, (g - 1) * 512 : (g + 1) * 512],
                                      res2)

                prev = energies_c(0)
                for g in range(1, NG):
                    cur = energies_c(g)
                    attend_c(g - 1, prev)
                    prev = cur
                attend_c(NG - 1, prev)

    return nc


def _prep_core_inputs(core, x, Wq, bq, Wk, bk, Wv, bv, gamma):
    b = core // 2
    p = core % 2
    g = float(np.asarray(gamma).reshape(-1)[0])
    qsl = slice(16 * p, 16 * p + 16)
    vsl = slice(128 * p, 128 * p + 128)

    import ml_dtypes
    bf = ml_dtypes.bfloat16

    # channel permutation: residual (output) channels first
    perm = np.concatenate([np.arange(128 * p, 128 * p + 128),
                           np.arange(128 * (1 - p), 128 * (1 - p) + 128)])

    wqk = np.zeros((C, 32), np.float32)
    wqk[:, 0:16] = Wq[qsl].T       # q head even(8) | q head odd(8)
    wqk[:, 16:32] = Wk[qsl].T
    wqk = wqk[perm].astype(bf)
    qkb = np.concatenate([bq[qsl], bk[qsl]]).reshape(32, 1).astype(np.float32)

    wv_eff = (g * Wv[vsl]).astype(np.float32)     # [128, 256]
    bv_eff = (g * bv[vsl]).astype(np.float32)
    wvt = np.zeros((C, 130), np.float32)
    wvt[:, 0:64] = wv_eff[0:64].T
    wvt[:, 65:129] = wv_eff[64:128].T
    wvt = wvt[perm].astype(bf)
    vbias = np.zeros((1, 130), np.float32)
    vbias[0, 0:64] = bv_eff[0:64]
    vbias[0, 64] = 1.0
    vbias[0, 65:129] = bv_eff[64:128]
    vbias[0, 129] = 1.0
    vbias_full = np.broadcast_to(vbias, (128, 130)).astype(bf)

    x2 = x[b].reshape(C, PIX)[perm]
    xpix = np.ascontiguousarray(
        x[b, vsl].transpose(2, 1, 0)).reshape(W, H * 128)
    return {
        "x_in": np.ascontiguousarray(x2).astype(bf),
        "x_pix": xpix.astype(bf),
        "wqkT": wqk,
        "qk_bias": qkb,
        "wvT": wvt,
        "vbias_full": np.ascontiguousarray(vbias_full),
    }


_NC_CACHE = None


def _get_nc():
    global _NC_CACHE
    if _NC_CACHE is None:
        nc = build_program()
        nc.compile()
        _NC_CACHE = nc
    return _NC_CACHE


def kernel(x, Wq, bq, Wk, bk, Wv, bv, gamma, _trace=False, _trace_kwargs=None):
    from concourse.bass_utils import run_bass_kernel_spmd

    nc = _get_nc()
    in_maps = [
        _prep_core_inputs(core, x, Wq, bq, Wk, bk, Wv, bv, gamma)
        for core in range(NCORES)
    ]
    res = run_bass_kernel_spmd(
        nc, in_maps, list(range(NCORES)), trace=_trace,
        **(_trace_kwargs or {}),
    )
    outp = np.empty((B, C, H, W), np.float32)
    for core in range(NCORES):
        b, p = core // 2, core % 2
        o = res.results[core]["out"].astype(np.float32)
        outp[b, 128 * p : 128 * p + 128] = (
            o.reshape(W, H, 128).transpose(2, 1, 0)
        )
    if _trace:
        kernel.last_results = res
    return outp


# revision 22
# speedup vs baseline: 1.4977x; 1.3519x over previous
"""CrissCrossAttention Trainium2 kernel (8 NeuronCores, data-parallel).

Problem: B=4, C=256, H=W=128, 4 heads. Per head: cq=8 q/k channels, cv=64
v channels. Row attention (over W per row) + column attention (over H per
column), outputs added with the CCNet spatial-transpose quirk, then
out = gamma*attn + x.

Sharding: 16 (batch, head) pairs over 8 cores -> each core handles
batch b = core//2 and head pair p = core%2 (global heads 2p, 2p+1).
Each core reads x[b] (all 256 channels, needed by the projections) and
produces output channels [128p : 128p+128] of batch b.

Host-side prep per core: x is bf16, channel-reordered so the residual
slice is rows 0-127 of x_in; weight rows permuted identically. A second
pixel-major copy x_pix[w, h*128+ch] feeds the residual add. Output is
bf16 pixel-major [w, h*128+ch]; host transposes/upcasts.

Core-local pipeline (pixels indexed pix = h*128 + w):
  - qk projection -> flat row-major fr[32, h*128+w] bf16; col-major
    fc[32, w*128+h] via incremental DVE permute. Bias fused into the
    PSUM evacuation.
  - band-packed operand stores for the PE (matmul operands must start at
    32-aligned partitions): q/k value for row h lives at partition
    32*(h%4)+c -> the 4 rows of a group occupy distinct PE row-groups and
    their K=8 energy matmuls run concurrently via tile_position (each into
    its own PSUM bank). Built with SBUF->SBUF DMAs: q/k issued
    incrementally during the projection loop; qc/kc after fc completes,
    spread over 3 issuing engines.
  - vT projection (pixel-major): vT[128w, 128h, 130c] bf16, channels =
    [64 head0 | 1 | 64 head1 | 1] with ones channels for the softmax
    denominator. Evacuation = DVE add of replicated bias.
  - vTc[h, w, c] = spatial transpose of vT via per-channel PE transposes.
  - Attention is software-pipelined one group deep so the PE never waits
    on the ACT exp: per group g, issue all 16 energy matmuls (dirs x
    heads x 4 rows, 4-way concurrent into pe's 4 banks), the two EXPs
    (one per dir, covering both heads), then the PREVIOUS group's 16
    value matmuls po (po's own 4 banks), reciprocal+multiply (DVE),
    CCNet combine t_row+t_col (GpSimd), residual add (DVE, pixel-major),
    DMA out. PSUM = pe 4 banks + po 4 banks, exact fit.
"""

import os
import numpy as np
from contextlib import ExitStack

import concourse.bass as bass
import concourse.bacc as bacc
import concourse.tile as tile
from concourse import mybir
from concourse.masks import make_identity

F32 = mybir.dt.float32
BF16 = mybir.dt.bfloat16

B, C, H, W = 4, 256, 128, 128
PIX = H * W            # 16384
CV = 64                # v channels per head
NCORES = 8
G = 4                  # rows per attention group (= PE row-group packing)
NG = H // G            # 32 groups


def build_program():
    nc = bacc.Bacc("TRN2", target_bir_lowering=False, debug=False,
                   num_devices=NCORES)

    x_in = nc.dram_tensor("x_in", [C, PIX], BF16, kind="ExternalInput")
    x_pix = nc.dram_tensor("x_pix", [W, H * 128], BF16, kind="ExternalInput")
    wqkT = nc.dram_tensor("wqkT", [C, 32], BF16, kind="ExternalInput")
    qk_bias = nc.dram_tensor("qk_bias", [32, 1], F32, kind="ExternalInput")
    wvT = nc.dram_tensor("wvT", [C, 130], BF16, kind="ExternalInput")
    vbias_full = nc.dram_tensor("vbias_full", [128, 130], BF16,
                                kind="ExternalInput")
    out = nc.dram_tensor("out", [W, H * 128], BF16, kind="ExternalOutput")

    with tile.TileContext(nc) as tc, ExitStack() as ctx:
        consts = ctx.enter_context(tc.tile_pool(name="consts", bufs=1))
        persist = ctx.enter_context(tc.tile_pool(name="persist", bufs=1))

        # constants / weights
        wqa = consts.tile([128, 32], BF16, tag="wqa")
        wqb = consts.tile([128, 32], BF16, tag="wqb")
        nc.sync.dma_start(wqa, wqkT[0:128, :])
        nc.sync.dma_start(wqb, wqkT[128:256, :])
        wva = consts.tile([128, 130], BF16, tag="wva")
        wvb = consts.tile([128, 130], BF16, tag="wvb")
        nc.sync.dma_start(wva, wvT[0:128, :])
        nc.sync.dma_start(wvb, wvT[128:256, :])
        qkb = consts.tile([32, 1], F32, tag="qkb")
        nc.sync.dma_start(qkb, qk_bias[:, :])
        vbias = consts.tile([128, 1, 130], BF16, tag="vbias")
        nc.sync.dma_start(vbias[:, 0, :], vbias_full[:, :])
        identb = consts.tile([128, 128], BF16, tag="identb")
        make_identity(nc, identb)

        # persistent activations
        # band-packed operand stores: partition 32*(h%4)+c, c<8
        q_sb = persist.tile([128, 2, H // 4, W], BF16, tag="q")    # 16 KiB
        k_sb = persist.tile([128, 2, H // 4, W], BF16, tag="k")    # 16 KiB
        qc_sb = persist.tile([128, 2, W // 4, H], BF16, tag="qc")  # 16 KiB
        kc_sb = persist.tile([128, 2, W // 4, H], BF16, tag="kc")  # 16 KiB
        # pixel-major value stores, channel innermost
        vT_sb = persist.tile([128, H, 130], BF16, tag="vT")        # 32.5 KiB
        vTc_sb = persist.tile([128, W, 130], BF16, tag="vTc")      # 32.5 KiB

        # ---------------- Phase B: projections ----------------
        with (
            tc.tile_pool(name="qkflat", bufs=1) as flatpool,
            tc.tile_pool(name="xchunk", bufs=3) as xpool,
            tc.tile_pool(name="pq", bufs=2, space="PSUM") as pqpool,
            tc.tile_pool(name="pv", bufs=4, space="PSUM") as pvpool,
        ):
            fr = flatpool.tile([32, PIX], BF16, tag="fr")  # [c, h*128+w]
            fc = flatpool.tile([32, PIX], BF16, tag="fc")  # [c, w*128+h]

            def bandpack_rows(dst_q, dst_k, src4, hb0, nhb, engs):
                # src4: [c, b, hb, w/h] view of fr or fc
                ei = 0
                for bb in range(4):
                    for hh in range(2):
                        eng = engs[ei % len(engs)]
                        ei += 1
                        eng.dma_start(
                            dst_q[32 * bb : 32 * bb + 8, hh, hb0 : hb0 + nhb, :],
                            src4[8 * hh : 8 * hh + 8, bb, hb0 : hb0 + nhb, :])
                        eng.dma_start(
                            dst_k[32 * bb : 32 * bb + 8, hh, hb0 : hb0 + nhb, :],
                            src4[16 + 8 * hh : 24 + 8 * hh, bb, hb0 : hb0 + nhb, :])

            src_r = fr[:, :].rearrange("c (hb b w) -> c b hb w", b=4, w=W)
            src_c = fc[:, :].rearrange("c (wb b h) -> c b wb h", b=4, h=H)

            CHUNK = 1024  # pixels per chunk = 8 rows
            NCH = PIX // CHUNK

            # x loads prefetched two chunks ahead to hide DMA latency
            xq = []

            def load_x(chi):
                c0 = chi * CHUNK
                eng = nc.sync if chi % 2 == 0 else nc.scalar
                xab = xpool.tile([128, CHUNK], BF16, tag="xab")
                xbb = xpool.tile([128, CHUNK], BF16, tag="xbb")
                eng.dma_start(xab, x_in[0:128, c0 : c0 + CHUNK])
                eng.dma_start(xbb, x_in[128:256, c0 : c0 + CHUNK])
                xq.append((xab, xbb))

            load_x(0)
            load_x(1)
            for chi in range(NCH):
                c0 = chi * CHUNK
                r0 = c0 // 128
                if chi + 2 < NCH:
                    load_x(chi + 2)
                xab, xbb = xq.pop(0)
                xav = xab[:, :].rearrange("p (r w) -> p r w", w=128)
                xbv = xbb[:, :].rearrange("p (r w) -> p r w", w=128)

                # qk projection, row-pixel order (matmul out <= 1 bank)
                pq = pqpool.tile([32, 2, 512], F32, tag="pq")
                for s in range(2):
                    nc.tensor.matmul(pq[:, s, :], wqa,
                                     xab[:, 512 * s : 512 * s + 512],
                                     start=True, stop=False,
                                     skip_group_check=True)
                    nc.tensor.matmul(pq[:, s, :], wqb,
                                     xbb[:, 512 * s : 512 * s + 512],
                                     start=False, stop=True,
                                     skip_group_check=True)
                nc.vector.tensor_scalar_add(
                    fr[:, c0 : c0 + CHUNK],
                    pq[:, :, :].rearrange("p s w -> p (s w)"), qkb)

                # vT projection: 2 rows per PSUM half-bank tile; bias is
                # added at evacuation (DVE), not via a PE matmul
                for s2 in range(4):
                    pv = pvpool.tile([128, 2, 130], F32, tag="pv")
                    for s3 in range(2):
                        srow = 2 * s2 + s3
                        nc.tensor.matmul(pv[:, s3, :], xav[:, srow, :], wva,
                                         start=(s3 == 0), stop=False,
                                         skip_group_check=True)
                        nc.tensor.matmul(pv[:, s3, :], xbv[:, srow, :], wvb,
                                         start=False, stop=(s3 == 1),
                                         skip_group_check=True)
                    nc.vector.tensor_tensor(
                        vT_sb[:, r0 + 2 * s2 : r0 + 2 * s2 + 2, :], pv,
                        vbias.to_broadcast((128, 2, 130)),
                        mybir.AluOpType.add)

                # col-major flat store slices: fc[:, :, h-slice] only needs
                # fr rows h-slice -> overlap the permute with projection
                if chi % 4 == 3:
                    hs = (chi // 4) * 32
                    frv = fr[:, :].rearrange("c (h w) -> c w h", w=W)
                    fcv = fc[:, :].rearrange("c (w h) -> c w h", h=H)
                    nc.vector.tensor_copy(fcv[:, :, hs : hs + 32],
                                          frv[:, :, hs : hs + 32])
                    # row-direction band-pack for the 8 groups just done
                    bandpack_rows(q_sb, k_sb, src_r, (chi // 4) * 8, 8,
                                  [nc.sync, nc.scalar])

            # column-direction band-pack (needs the full fc); spread over
            # three issuing engines
            bandpack_rows(qc_sb, kc_sb, src_c, 0, 32,
                          [nc.sync, nc.scalar, nc.gpsimd])

        # ---------------- Phase B2: vTc via PE transposes ----------------
        # vT[w, h, c] -> vTc[h, w, c]; per channel, batched 4 per bank.
        with tc.tile_pool(name="ptr", bufs=2, space="PSUM") as ptrpool:
            for cb in range(33):
                nch = min(4, 130 - cb * 4)
                ptr = ptrpool.tile([128, 4, 128], BF16, tag="ptr")
                for cj in range(nch):
                    cch = cb * 4 + cj
                    nc.tensor.matmul(ptr[:, cj, :], vT_sb[:, :, cch], identb,
                                     start=True, stop=True, is_transpose=True)
                dst = vTc_sb[:, :, cb * 4 : cb * 4 + nch]
                src = ptr[:, 0:nch, :].rearrange("p c w -> p w c")
                if cb % 2 == 0:
                    nc.vector.tensor_copy(dst, src)
                else:
                    nc.scalar.copy(dst, src)

        # ---------------- Phase C: attention (pipelined) ----------------
        with (
            tc.tile_pool(name="pe", bufs=1, space="PSUM") as pepool,
            tc.tile_pool(name="po", bufs=1, space="PSUM") as popool,
            tc.tile_pool(name="pt", bufs=4) as ptpool,
            tc.tile_pool(name="tt", bufs=3) as tpool,
            tc.tile_pool(name="au", bufs=2) as aupool,
            tc.tile_pool(name="rc", bufs=4) as rcpool,
            tc.tile_pool(name="io", bufs=3) as iopool,
        ):
            def energies(g):
                # all 16 energy matmuls into pe's 4 banks (bank = row j),
                # then one EXP per direction covering both heads
                pe = pepool.tile([128, G, 512], F32, tag="pe")
                for d in range(2):
                    qs = q_sb if d == 0 else qc_sb
                    ks = k_sb if d == 0 else kc_sb
                    for hh in range(2):
                        for j in range(G):
                            nc.tensor.matmul(
                                pe[:, j, 256 * d + 128 * hh :
                                   256 * d + 128 * hh + 128],
                                ks[32 * j : 32 * j + 8, hh, g, :],
                                qs[32 * j : 32 * j + 8, hh, g, :],
                                start=True, stop=True,
                                tile_position=(32 * j, 0),
                                skip_group_check=True,
                            )
                # one EXP covering both directions and heads (contiguous)
                pT = ptpool.tile([128, G, 512], BF16, tag="pt")
                nc.scalar.activation(pT, pe[:, :, :],
                                     mybir.ActivationFunctionType.Exp)
                return pT

            def attend(g, pT):
                po = popool.tile([128, 2, 2, G, 128], F32, tag="po")
                for d in range(2):
                    vs = vT_sb if d == 0 else vTc_sb
                    for hh in range(2):
                        for j in range(G):
                            i = g * G + j
                            nc.tensor.matmul(
                                po[:, d, hh, j, 0:65],
                                pT[:, j, 256 * d + 128 * hh :
                                   256 * d + 128 * hh + 128],
                                vs[:, i, 65 * hh : 65 * hh + 65],
                                start=True, stop=True,
                                skip_group_check=True,
                            )
                # single reciprocal + multiply over both dirs, po-native
                # (d, hh, j, c) order; host decodes the channel order
                til = tpool.tile([128, 2, 2, G, CV], BF16, tag="t")
                rec = rcpool.tile([128, 2, 2, G, 1], F32, tag="rc")
                nc.vector.reciprocal(rec, po[:, :, :, :, 64:65])
                nc.vector.tensor_tensor(
                    til, po[:, :, :, :, 0:64],
                    rec.to_broadcast((128, 2, 2, G, CV)),
                    mybir.AluOpType.mult,
                )
                au = aupool.tile([128, 2, G, CV], BF16, tag="au")
                nc.gpsimd.tensor_tensor(au, til[:, 0, :, :, :],
                                        til[:, 1, :, :, :],
                                        mybir.AluOpType.add)
                eng = nc.sync if g % 2 == 0 else nc.scalar
                xpg = iopool.tile([128, G * 128], BF16, tag="xpg")
                eng.dma_start(xpg, x_pix[:, g * 512 : (g + 1) * 512])
                res = iopool.tile([128, G * 128], BF16, tag="res")
                nc.vector.tensor_tensor(
                    res, au[:, :, :, :].rearrange("p hh g c -> p (hh g c)"),
                    xpg, mybir.AluOpType.add)
                eng.dma_start(out[:, g * 512 : (g + 1) * 512], res)

            prev = energies(0)
            for g in range(1, NG):
                cur = energies(g)
                attend(g - 1, prev)
                prev = cur
            attend(NG - 1, prev)

    return nc


def _prep_core_inputs(core, x, Wq, bq, Wk, bk, Wv, bv, gamma):
    b = core // 2
    p = core % 2
    g = float(np.asarray(gamma).reshape(-1)[0])
    qsl = slice(16 * p, 16 * p + 16)
    vsl = slice(128 * p, 128 * p + 128)

    import ml_dtypes
    bf = ml_dtypes.bfloat16

    # channel permutation: residual (output) channels first
    perm = np.concatenate([np.arange(128 * p, 128 * p + 128),
                           np.arange(128 * (1 - p), 128 * (1 - p) + 128)])

    wqk = np.zeros((C, 32), np.float32)
    wqk[:, 0:16] = Wq[qsl].T       # q head even(8) | q head odd(8)
    wqk[:, 16:32] = Wk[qsl].T
    wqk = wqk[perm].astype(bf)
    qkb = np.concatenate([bq[qsl], bk[qsl]]).reshape(32, 1).astype(np.float32)

    wv_eff = (g * Wv[vsl]).astype(np.float32)     # [128, 256]
    bv_eff = (g * bv[vsl]).astype(np.float32)
    wvt = np.zeros((C, 130), np.float32)
    wvt[:, 0:64] = wv_eff[0:64].T
    wvt[:, 65:129] = wv_eff[64:128].T
    wvt = wvt[perm].astype(bf)
    vbias = np.zeros((1, 130), np.float32)
    vbias[0, 0:64] = bv_eff[0:64]
    vbias[0, 64] = 1.0
    vbias[0, 65:129] = bv_eff[64:128]
    vbias[0, 129] = 1.0
    vbias_full = np.broadcast_to(vbias, (128, 130)).astype(bf)

    x2 = x[b].reshape(C, PIX)[perm]
    # pixel-major residual: [w][h][ch] so per-group loads are contiguous
    # device group layout per 4-row group: (hh, j, cv); build
    # x_pix[w, g*512 + hh*256 + j*64 + cv] = x[ch=hh*64+cv, h=4g+j, w]
    xpix = np.ascontiguousarray(
        x[b, vsl].reshape(2, CV, NG, G, W).transpose(4, 2, 0, 3, 1)
    ).reshape(W, H * 128)
    return {
        "x_in": np.ascontiguousarray(x2).astype(bf),
        "x_pix": xpix.astype(bf),
        "wqkT": wqk,
        "qk_bias": qkb,
        "wvT": wvt,
        "vbias_full": np.ascontiguousarray(vbias_full),
    }


_NC_CACHE = None


def _get_nc():
    global _NC_CACHE
    if _NC_CACHE is None:
        nc = build_program()
        nc.compile()
        _NC_CACHE = nc
    return _NC_CACHE


def kernel(x, Wq, bq, Wk, bk, Wv, bv, gamma, _trace=False, _trace_kwargs=None):
    from concourse.bass_utils import run_bass_kernel_spmd

    nc = _get_nc()
    in_maps = [
        _prep_core_inputs(core, x, Wq, bq, Wk, bk, Wv, bv, gamma)
        for core in range(NCORES)
    ]
    res = run_bass_kernel_spmd(
        nc, in_maps, list(range(NCORES)), trace=_trace,
        **(_trace_kwargs or {}),
    )
    outp = np.empty((B, C, H, W), np.float32)
    for core in range(NCORES):
        b, p = core // 2, core % 2
        o = res.results[core]["out"].astype(np.float32)
        # out[w, h*128+ch] -> [ch, h, w]
        outp[b, 128 * p : 128 * p + 128] = (
            o.reshape(W, NG, 2, G, CV).transpose(2, 4, 1, 3, 0).reshape(
                128, H, W)
        )
    if _trace:
        kernel.last_results = res
    return outp


# revision 24
# speedup vs baseline: 1.5263x; 1.0191x over previous
"""CrissCrossAttention Trainium2 kernel (8 NeuronCores, data-parallel).

Problem: B=4, C=256, H=W=128, 4 heads. Per head: cq=8 q/k channels, cv=64
v channels. Row attention (over W per row) + column attention (over H per
column), outputs added with the CCNet spatial-transpose quirk, then
out = gamma*attn + x.

Sharding: 16 (batch, head) pairs over 8 cores -> each core handles
batch b = core//2 and head pair p = core%2 (global heads 2p, 2p+1).
Each core reads x[b] (all 256 channels, needed by the projections) and
produces output channels [128p : 128p+128] of batch b.

Host-side prep per core: x is bf16, channel-reordered so the residual
slice is rows 0-127 of x_in; weight rows permuted identically. A second
pixel-major copy x_pix[w, h*128+ch] feeds the residual add. Output is
bf16 pixel-major [w, h*128+ch]; host transposes/upcasts.

Core-local pipeline (pixels indexed pix = h*128 + w):
  - qk projection -> flat row-major fr[32, h*128+w] bf16; col-major
    fc[32, w*128+h] via incremental DVE permute. Bias fused into the
    PSUM evacuation.
  - band-packed operand stores for the PE (matmul operands must start at
    32-aligned partitions): q/k value for row h lives at partition
    32*(h%4)+c -> the 4 rows of a group occupy distinct PE row-groups and
    their K=8 energy matmuls run concurrently via tile_position (each into
    its own PSUM bank). Built with SBUF->SBUF DMAs: q/k issued
    incrementally during the projection loop; qc/kc after fc completes,
    spread over 3 issuing engines.
  - vT projection (pixel-major): vT[128w, 128h, 130c] bf16, channels =
    [64 head0 | 1 | 64 head1 | 1] with ones channels for the softmax
    denominator. Evacuation = DVE add of replicated bias.
  - vTc[h, w, c] = spatial transpose of vT via per-channel PE transposes.
  - Attention is software-pipelined one group deep so the PE never waits
    on the ACT exp: per group g, issue all 16 energy matmuls (dirs x
    heads x 4 rows, 4-way concurrent into pe's 4 banks), the two EXPs
    (one per dir, covering both heads), then the PREVIOUS group's 16
    value matmuls po (po's own 4 banks), reciprocal+multiply (DVE),
    CCNet combine t_row+t_col (GpSimd), residual add (DVE, pixel-major),
    DMA out. PSUM = pe 4 banks + po 4 banks, exact fit.
"""

import os
import numpy as np
from contextlib import ExitStack

import concourse.bass as bass
import concourse.bacc as bacc
import concourse.tile as tile
from concourse import mybir
from concourse.masks import make_identity

F32 = mybir.dt.float32
BF16 = mybir.dt.bfloat16

B, C, H, W = 4, 256, 128, 128
PIX = H * W            # 16384
CV = 64                # v channels per head
NCORES = 8
G = 4                  # rows per attention group (= PE row-group packing)
NG = H // G            # 32 groups


def build_program():
    nc = bacc.Bacc("TRN2", target_bir_lowering=False, debug=False,
                   num_devices=NCORES)

    x_in = nc.dram_tensor("x_in", [C, PIX], BF16, kind="ExternalInput")
    x_pix = nc.dram_tensor("x_pix", [W, H * 128], BF16, kind="ExternalInput")
    wqkT = nc.dram_tensor("wqkT", [C, 32], BF16, kind="ExternalInput")
    qk_bias = nc.dram_tensor("qk_bias", [32, 1], F32, kind="ExternalInput")
    wvT = nc.dram_tensor("wvT", [C, 130], BF16, kind="ExternalInput")
    vbias_full = nc.dram_tensor("vbias_full", [128, 130], BF16,
                                kind="ExternalInput")
    out = nc.dram_tensor("out", [W, H * 128], BF16, kind="ExternalOutput")

    with tile.TileContext(nc) as tc, ExitStack() as ctx:
        consts = ctx.enter_context(tc.tile_pool(name="consts", bufs=1))
        persist = ctx.enter_context(tc.tile_pool(name="persist", bufs=1))

        # constants / weights
        wqa = consts.tile([128, 32], BF16, tag="wqa")
        wqb = consts.tile([128, 32], BF16, tag="wqb")
        nc.sync.dma_start(wqa, wqkT[0:128, :])
        nc.sync.dma_start(wqb, wqkT[128:256, :])
        wva = consts.tile([128, 130], BF16, tag="wva")
        wvb = consts.tile([128, 130], BF16, tag="wvb")
        nc.sync.dma_start(wva, wvT[0:128, :])
        nc.sync.dma_start(wvb, wvT[128:256, :])
        qkb = consts.tile([32, 1], F32, tag="qkb")
        nc.sync.dma_start(qkb, qk_bias[:, :])
        vbias = consts.tile([128, 1, 130], BF16, tag="vbias")
        nc.sync.dma_start(vbias[:, 0, :], vbias_full[:, :])
        identb = consts.tile([128, 128], BF16, tag="identb")
        make_identity(nc, identb)

        # persistent activations
        # band-packed operand stores: partition 32*(h%4)+c, c<8
        q_sb = persist.tile([128, 2, H // 4, W], BF16, tag="q")    # 16 KiB
        k_sb = persist.tile([128, 2, H // 4, W], BF16, tag="k")    # 16 KiB
        qc_sb = persist.tile([128, 2, W // 4, H], BF16, tag="qc")  # 16 KiB
        kc_sb = persist.tile([128, 2, W // 4, H], BF16, tag="kc")  # 16 KiB
        # pixel-major value stores, channel innermost
        vT_sb = persist.tile([128, H, 130], BF16, tag="vT")        # 32.5 KiB
        vTc_sb = persist.tile([128, W, 130], BF16, tag="vTc")      # 32.5 KiB

        # ---------------- Phase B: projections ----------------
        with (
            tc.tile_pool(name="qkflat", bufs=1) as flatpool,
            tc.tile_pool(name="xchunk", bufs=3) as xpool,
            tc.tile_pool(name="pq", bufs=2, space="PSUM") as pqpool,
            tc.tile_pool(name="pv", bufs=4, space="PSUM") as pvpool,
        ):
            fr = flatpool.tile([32, PIX], BF16, tag="fr")  # [c, h*128+w]
            fc = flatpool.tile([32, PIX], BF16, tag="fc")  # [c, w*128+h]

            def bandpack_rows(dst_q, dst_k, src4, hb0, nhb, engs):
                # src4: [c, b, hb, w/h] view of fr or fc
                ei = 0
                for bb in range(4):
                    for hh in range(2):
                        eng = engs[ei % len(engs)]
                        ei += 1
                        eng.dma_start(
                            dst_q[32 * bb : 32 * bb + 8, hh, hb0 : hb0 + nhb, :],
                            src4[8 * hh : 8 * hh + 8, bb, hb0 : hb0 + nhb, :])
                        eng.dma_start(
                            dst_k[32 * bb : 32 * bb + 8, hh, hb0 : hb0 + nhb, :],
                            src4[16 + 8 * hh : 24 + 8 * hh, bb, hb0 : hb0 + nhb, :])

            src_r = fr[:, :].rearrange("c (hb b w) -> c b hb w", b=4, w=W)
            src_c = fc[:, :].rearrange("c (wb b h) -> c b wb h", b=4, h=H)

            CHUNK = 1024  # pixels per chunk = 8 rows
            NCH = PIX // CHUNK

            # x loads prefetched two chunks ahead to hide DMA latency
            xq = []

            def load_x(chi):
                c0 = chi * CHUNK
                eng = nc.sync if chi % 2 == 0 else nc.scalar
                xab = xpool.tile([128, CHUNK], BF16, tag="xab")
                xbb = xpool.tile([128, CHUNK], BF16, tag="xbb")
                eng.dma_start(xab, x_in[0:128, c0 : c0 + CHUNK])
                eng.dma_start(xbb, x_in[128:256, c0 : c0 + CHUNK])
                xq.append((xab, xbb))

            load_x(0)
            load_x(1)
            for chi in range(NCH):
                c0 = chi * CHUNK
                r0 = c0 // 128
                if chi + 2 < NCH:
                    load_x(chi + 2)
                xab, xbb = xq.pop(0)
                xav = xab[:, :].rearrange("p (r w) -> p r w", w=128)
                xbv = xbb[:, :].rearrange("p (r w) -> p r w", w=128)

                # qk projection, row-pixel order (matmul out <= 1 bank)
                pq = pqpool.tile([32, 2, 512], F32, tag="pq")
                for s in range(2):
                    nc.tensor.matmul(pq[:, s, :], wqa,
                                     xab[:, 512 * s : 512 * s + 512],
                                     start=True, stop=False,
                                     skip_group_check=True)
                    nc.tensor.matmul(pq[:, s, :], wqb,
                                     xbb[:, 512 * s : 512 * s + 512],
                                     start=False, stop=True,
                                     skip_group_check=True)
                nc.vector.tensor_scalar_add(
                    fr[:, c0 : c0 + CHUNK],
                    pq[:, :, :].rearrange("p s w -> p (s w)"), qkb)

                # vT projection: 2 rows per PSUM half-bank tile; bias is
                # added at evacuation (DVE), not via a PE matmul
                for s2 in range(4):
                    pv = pvpool.tile([128, 2, 130], F32, tag="pv")
                    for s3 in range(2):
                        srow = 2 * s2 + s3
                        nc.tensor.matmul(pv[:, s3, :], xav[:, srow, :], wva,
                                         start=(s3 == 0), stop=False,
                                         skip_group_check=True)
                        nc.tensor.matmul(pv[:, s3, :], xbv[:, srow, :], wvb,
                                         start=False, stop=(s3 == 1),
                                         skip_group_check=True)
                    nc.vector.tensor_tensor(
                        vT_sb[:, r0 + 2 * s2 : r0 + 2 * s2 + 2, :], pv,
                        vbias.to_broadcast((128, 2, 130)),
                        mybir.AluOpType.add)

                # col-major flat store slices: fc[:, :, h-slice] only needs
                # fr rows h-slice -> overlap the permute with projection
                if chi % 4 == 3:
                    hs = (chi // 4) * 32
                    frv = fr[:, :].rearrange("c (h w) -> c w h", w=W)
                    fcv = fc[:, :].rearrange("c (w h) -> c w h", h=H)
                    nc.vector.tensor_copy(fcv[:, :, hs : hs + 32],
                                          frv[:, :, hs : hs + 32])
                    # row-direction band-pack for the 8 groups just done
                    bandpack_rows(q_sb, k_sb, src_r, (chi // 4) * 8, 8,
                                  [nc.sync, nc.scalar])

            # column-direction band-pack (needs the full fc); spread over
            # three issuing engines
            bandpack_rows(qc_sb, kc_sb, src_c, 0, 32,
                          [nc.sync, nc.scalar, nc.gpsimd])

        # ---------------- Phase B2: vTc via PE transposes ----------------
        # vT[w, h, c] -> vTc[h, w, c]; per channel, batched 4 per bank.
        with tc.tile_pool(name="ptr", bufs=2, space="PSUM") as ptrpool:
            for cb in range(33):
                nch = min(4, 130 - cb * 4)
                ptr = ptrpool.tile([128, 4, 128], BF16, tag="ptr")
                for cj in range(nch):
                    cch = cb * 4 + cj
                    nc.tensor.matmul(ptr[:, cj, :], vT_sb[:, :, cch], identb,
                                     start=True, stop=True, is_transpose=True)
                dst = vTc_sb[:, :, cb * 4 : cb * 4 + nch]
                src = ptr[:, 0:nch, :].rearrange("p c w -> p w c")
                if cb % 2 == 0:
                    nc.vector.tensor_copy(dst, src)
                else:
                    nc.scalar.copy(dst, src)

        # ---------------- Phase C: attention (pipelined) ----------------
        with (
            tc.tile_pool(name="pe", bufs=1, space="PSUM") as pepool,
            tc.tile_pool(name="po", bufs=2, space="PSUM") as popool,
            tc.tile_pool(name="pt", bufs=4) as ptpool,
            tc.tile_pool(name="tt", bufs=3) as tpool,
            tc.tile_pool(name="au", bufs=2) as aupool,
            tc.tile_pool(name="rc", bufs=4) as rcpool,
            tc.tile_pool(name="io", bufs=3) as iopool,
        ):
            def energies(g):
                # 16 energy matmuls into pe's 4 banks (bank = row j); one
                # EXP per direction so subtile WAR frees pe's d0 half for
                # the next group while d1's EXP still runs
                pe = pepool.tile([128, G, 512], F32, tag="pe")
                pTs = []
                for d in range(2):
                    qs = q_sb if d == 0 else qc_sb
                    ks = k_sb if d == 0 else kc_sb
                    for hh in range(2):
                        for j in range(G):
                            nc.tensor.matmul(
                                pe[:, j, 256 * d + 128 * hh :
                                   256 * d + 128 * hh + 128],
                                ks[32 * j : 32 * j + 8, hh, g, :],
                                qs[32 * j : 32 * j + 8, hh, g, :],
                                start=True, stop=True,
                                tile_position=(32 * j, 0),
                                skip_group_check=True,
                            )
                    pT = ptpool.tile([128, G, 256], BF16, tag="pt")
                    nc.scalar.activation(
                        pT, pe[:, :, 256 * d : 256 * d + 256],
                        mybir.ActivationFunctionType.Exp)
                    pTs.append(pT)
                return pTs

            def attend(g, pTs):
                # per-direction po tiles, double-buffered: po(g) never
                # waits on the previous group's reciprocal/multiply
                tils = []
                for d in range(2):
                    vs = vT_sb if d == 0 else vTc_sb
                    po = popool.tile([128, 2, G, 128], F32, tag="po")
                    for hh in range(2):
                        for j in range(G):
                            i = g * G + j
                            nc.tensor.matmul(
                                po[:, hh, j, 0:65],
                                pTs[d][:, j, 128 * hh : 128 * hh + 128],
                                vs[:, i, 65 * hh : 65 * hh + 65],
                                start=True, stop=True,
                                skip_group_check=True,
                            )
                    # po-native (hh, j, c) order; host decodes
                    til = tpool.tile([128, 2, G, CV], BF16, tag="t")
                    rec = rcpool.tile([128, 2, G, 1], F32, tag="rc")
                    nc.vector.reciprocal(rec, po[:, :, :, 64:65])
                    nc.vector.tensor_tensor(
                        til, po[:, :, :, 0:64],
                        rec.to_broadcast((128, 2, G, CV)),
                        mybir.AluOpType.mult,
                    )
                    tils.append(til)
                au = aupool.tile([128, 2, G, CV], BF16, tag="au")
                nc.gpsimd.tensor_tensor(au, tils[0][:, :, :, :],
                                        tils[1][:, :, :, :],
                                        mybir.AluOpType.add)
                eng = nc.sync if g % 2 == 0 else nc.scalar
                xpg = iopool.tile([128, G * 128], BF16, tag="xpg")
                eng.dma_start(xpg, x_pix[:, g * 512 : (g + 1) * 512])
                res = iopool.tile([128, G * 128], BF16, tag="res")
                nc.vector.tensor_tensor(
                    res, au[:, :, :, :].rearrange("p hh g c -> p (hh g c)"),
                    xpg, mybir.AluOpType.add)
                eng.dma_start(out[:, g * 512 : (g + 1) * 512], res)

            prev = energies(0)
            for g in range(1, NG):
                cur = energies(g)
                attend(g - 1, prev)
                prev = cur
            attend(NG - 1, prev)

    return nc


def _prep_core_inputs(core, x, Wq, bq, Wk, bk, Wv, bv, gamma):
    b = core // 2
    p = core % 2
    g = float(np.asarray(gamma).reshape(-1)[0])
    qsl = slice(16 * p, 16 * p + 16)
    vsl = slice(128 * p, 128 * p + 128)

    import ml_dtypes
    bf = ml_dtypes.bfloat16

    # channel permutation: residual (output) channels first
    perm = np.concatenate([np.arange(128 * p, 128 * p + 128),
                           np.arange(128 * (1 - p), 128 * (1 - p) + 128)])

    wqk = np.zeros((C, 32), np.float32)
    wqk[:, 0:16] = Wq[qsl].T       # q head even(8) | q head odd(8)
    wqk[:, 16:32] = Wk[qsl].T
    wqk = wqk[perm].astype(bf)
    qkb = np.concatenate([bq[qsl], bk[qsl]]).reshape(32, 1).astype(np.float32)

    wv_eff = (g * Wv[vsl]).astype(np.float32)     # [128, 256]
    bv_eff = (g * bv[vsl]).astype(np.float32)
    wvt = np.zeros((C, 130), np.float32)
    wvt[:, 0:64] = wv_eff[0:64].T
    wvt[:, 65:129] = wv_eff[64:128].T
    wvt = wvt[perm].astype(bf)
    vbias = np.zeros((1, 130), np.float32)
    vbias[0, 0:64] = bv_eff[0:64]
    vbias[0, 64] = 1.0
    vbias[0, 65:129] = bv_eff[64:128]
    vbias[0, 129] = 1.0
    vbias_full = np.broadcast_to(vbias, (128, 130)).astype(bf)

    x2 = x[b].reshape(C, PIX)[perm]
    # pixel-major residual: [w][h][ch] so per-group loads are contiguous
    # device group layout per 4-row group: (hh, j, cv); build
    # x_pix[w, g*512 + hh*256 + j*64 + cv] = x[ch=hh*64+cv, h=4g+j, w]
    xpix = np.ascontiguousarray(
        x[b, vsl].reshape(2, CV, NG, G, W).transpose(4, 2, 0, 3, 1)
    ).reshape(W, H * 128)
    return {
        "x_in": np.ascontiguousarray(x2).astype(bf),
        "x_pix": xpix.astype(bf),
        "wqkT": wqk,
        "qk_bias": qkb,
        "wvT": wvt,
        "vbias_full": np.ascontiguousarray(vbias_full),
    }


_NC_CACHE = None


def _get_nc():
    global _NC_CACHE
    if _NC_CACHE is None:
        nc = build_program()
        nc.compile()
        _NC_CACHE = nc
    return _NC_CACHE


def kernel(x, Wq, bq, Wk, bk, Wv, bv, gamma, _trace=False, _trace_kwargs=None):
    from concourse.bass_utils import run_bass_kernel_spmd

    nc = _get_nc()
    in_maps = [
        _prep_core_inputs(core, x, Wq, bq, Wk, bk, Wv, bv, gamma)
        for core in range(NCORES)
    ]
    res = run_bass_kernel_spmd(
        nc, in_maps, list(range(NCORES)), trace=_trace,
        **(_trace_kwargs or {}),
    )
    outp = np.empty((B, C, H, W), np.float32)
    for core in range(NCORES):
        b, p = core // 2, core % 2
        o = res.results[core]["out"].astype(np.float32)
        # out[w, h*128+ch] -> [ch, h, w]
        outp[b, 128 * p : 128 * p + 128] = (
            o.reshape(W, NG, 2, G, CV).transpose(2, 4, 1, 3, 0).reshape(
                128, H, W)
        )
    if _trace:
        kernel.last_results = res
    return outp


# revision 26
# speedup vs baseline: 1.5630x; 1.0240x over previous
"""CrissCrossAttention Trainium2 kernel (8 NeuronCores, data-parallel).

Problem: B=4, C=256, H=W=128, 4 heads. Per head: cq=8 q/k channels, cv=64
v channels. Row attention (over W per row) + column attention (over H per
column), outputs added with the CCNet spatial-transpose quirk, then
out = gamma*attn + x.

Sharding: 16 (batch, head) pairs over 8 cores -> each core handles
batch b = core//2 and head pair p = core%2 (global heads 2p, 2p+1).
Each core reads x[b] (all 256 channels, needed by the projections) and
produces output channels [128p : 128p+128] of batch b.

Host-side prep per core: x is bf16, channel-reordered so the residual
slice is rows 0-127 of x_in; weight rows permuted identically. A second
pixel-major copy x_pix[w, h*128+ch] feeds the residual add. Output is
bf16 pixel-major [w, h*128+ch]; host transposes/upcasts.

Core-local pipeline (pixels indexed pix = h*128 + w):
  - qk projection -> flat row-major fr[32, h*128+w] bf16; col-major
    fc[32, w*128+h] via incremental DVE permute. Bias fused into the
    PSUM evacuation.
  - band-packed operand stores for the PE (matmul operands must start at
    32-aligned partitions): q/k value for row h lives at partition
    32*(h%4)+c -> the 4 rows of a group occupy distinct PE row-groups and
    their K=8 energy matmuls run concurrently via tile_position (each into
    its own PSUM bank). Built with SBUF->SBUF DMAs: q/k issued
    incrementally during the projection loop; qc/kc after fc completes,
    spread over 3 issuing engines.
  - vT projection (pixel-major): vT[128w, 128h, 130c] bf16, channels =
    [64 head0 | 1 | 64 head1 | 1] with ones channels for the softmax
    denominator. Evacuation = DVE add of replicated bias.
  - vTc[h, w, c] = spatial transpose of vT via per-channel PE transposes.
  - Attention is software-pipelined one group deep so the PE never waits
    on the ACT exp: per group g, issue all 16 energy matmuls (dirs x
    heads x 4 rows, 4-way concurrent into pe's 4 banks), the two EXPs
    (one per dir, covering both heads), then the PREVIOUS group's 16
    value matmuls po (po's own 4 banks), reciprocal+multiply (DVE),
    CCNet combine t_row+t_col (GpSimd), residual add (DVE, pixel-major),
    DMA out. PSUM = pe 4 banks + po 4 banks, exact fit.
"""

import os
import numpy as np
from contextlib import ExitStack

import concourse.bass as bass
import concourse.bacc as bacc
import concourse.tile as tile
from concourse import mybir
from concourse.masks import make_identity

F32 = mybir.dt.float32
BF16 = mybir.dt.bfloat16

B, C, H, W = 4, 256, 128, 128
PIX = H * W            # 16384
CV = 64                # v channels per head
NCORES = 8
G = 4                  # rows per attention group (= PE row-group packing)
NG = H // G            # 32 groups


def build_program():
    nc = bacc.Bacc("TRN2", target_bir_lowering=False, debug=False,
                   num_devices=NCORES)

    x_in = nc.dram_tensor("x_in", [C, PIX], BF16, kind="ExternalInput")
    x_pix = nc.dram_tensor("x_pix", [W, H * 128], BF16, kind="ExternalInput")
    wqkT = nc.dram_tensor("wqkT", [C, 32], BF16, kind="ExternalInput")
    qk_bias = nc.dram_tensor("qk_bias", [32, 1], F32, kind="ExternalInput")
    wvT = nc.dram_tensor("wvT", [C, 130], BF16, kind="ExternalInput")
    vbias_full = nc.dram_tensor("vbias_full", [128, 130], BF16,
                                kind="ExternalInput")
    out = nc.dram_tensor("out", [W, H * 128], BF16, kind="ExternalOutput")

    with tile.TileContext(nc) as tc, ExitStack() as ctx:
        consts = ctx.enter_context(tc.tile_pool(name="consts", bufs=1))
        persist = ctx.enter_context(tc.tile_pool(name="persist", bufs=1))

        # constants / weights
        wqa = consts.tile([128, 32], BF16, tag="wqa")
        wqb = consts.tile([128, 32], BF16, tag="wqb")
        nc.sync.dma_start(wqa, wqkT[0:128, :])
        nc.sync.dma_start(wqb, wqkT[128:256, :])
        wva = consts.tile([128, 130], BF16, tag="wva")
        wvb = consts.tile([128, 130], BF16, tag="wvb")
        nc.sync.dma_start(wva, wvT[0:128, :])
        nc.sync.dma_start(wvb, wvT[128:256, :])
        qkb = consts.tile([32, 1], F32, tag="qkb")
        nc.sync.dma_start(qkb, qk_bias[:, :])
        vbias = consts.tile([128, 1, 130], BF16, tag="vbias")
        nc.sync.dma_start(vbias[:, 0, :], vbias_full[:, :])
        identb = consts.tile([128, 128], BF16, tag="identb")
        make_identity(nc, identb)

        # persistent activations
        # band-packed operand stores: partition 32*(h%4)+c, c<8
        q_sb = persist.tile([128, 2, H // 4, W], BF16, tag="q")    # 16 KiB
        k_sb = persist.tile([128, 2, H // 4, W], BF16, tag="k")    # 16 KiB
        qc_sb = persist.tile([128, 2, W // 4, H], BF16, tag="qc")  # 16 KiB
        kc_sb = persist.tile([128, 2, W // 4, H], BF16, tag="kc")  # 16 KiB
        # pixel-major value stores, channel innermost
        vT_sb = persist.tile([128, H, 130], BF16, tag="vT")        # 32.5 KiB
        vTc_sb = persist.tile([128, W, 130], BF16, tag="vTc")      # 32.5 KiB

        # ---------------- Phase B: projections ----------------
        with (
            tc.tile_pool(name="qkflat", bufs=1) as flatpool,
            tc.tile_pool(name="xchunk", bufs=3) as xpool,
            tc.tile_pool(name="pq", bufs=2, space="PSUM") as pqpool,
            tc.tile_pool(name="pv", bufs=4, space="PSUM") as pvpool,
        ):
            fr = flatpool.tile([32, PIX], BF16, tag="fr")  # [c, h*128+w]
            fc = flatpool.tile([32, PIX], BF16, tag="fc")  # [c, w*128+h]

            def bandpack_rows(dst_q, dst_k, src4, hb0, nhb, engs):
                # src4: [c, b, hb, w/h] view of fr or fc
                ei = 0
                for bb in range(4):
                    for hh in range(2):
                        eng = engs[ei % len(engs)]
                        ei += 1
                        eng.dma_start(
                            dst_q[32 * bb : 32 * bb + 8, hh, hb0 : hb0 + nhb, :],
                            src4[8 * hh : 8 * hh + 8, bb, hb0 : hb0 + nhb, :])
                        eng.dma_start(
                            dst_k[32 * bb : 32 * bb + 8, hh, hb0 : hb0 + nhb, :],
                            src4[16 + 8 * hh : 24 + 8 * hh, bb, hb0 : hb0 + nhb, :])

            src_r = fr[:, :].rearrange("c (hb b w) -> c b hb w", b=4, w=W)
            src_c = fc[:, :].rearrange("c (wb b h) -> c b wb h", b=4, h=H)

            CHUNK = 1024  # pixels per chunk = 8 rows
            NCH = PIX // CHUNK

            # x loads prefetched two chunks ahead to hide DMA latency
            xq = []

            def load_x(chi):
                c0 = chi * CHUNK
                eng = nc.sync if chi % 2 == 0 else nc.scalar
                xab = xpool.tile([128, CHUNK], BF16, tag="xab")
                xbb = xpool.tile([128, CHUNK], BF16, tag="xbb")
                eng.dma_start(xab, x_in[0:128, c0 : c0 + CHUNK])
                eng.dma_start(xbb, x_in[128:256, c0 : c0 + CHUNK])
                xq.append((xab, xbb))

            load_x(0)
            load_x(1)
            for chi in range(NCH):
                c0 = chi * CHUNK
                r0 = c0 // 128
                if chi + 2 < NCH:
                    load_x(chi + 2)
                xab, xbb = xq.pop(0)
                xav = xab[:, :].rearrange("p (r w) -> p r w", w=128)
                xbv = xbb[:, :].rearrange("p (r w) -> p r w", w=128)

                # qk projection, row-pixel order (matmul out <= 1 bank)
                pq = pqpool.tile([32, 2, 512], F32, tag="pq")
                for s in range(2):
                    nc.tensor.matmul(pq[:, s, :], wqa,
                                     xab[:, 512 * s : 512 * s + 512],
                                     start=True, stop=False,
                                     skip_group_check=True)
                    nc.tensor.matmul(pq[:, s, :], wqb,
                                     xbb[:, 512 * s : 512 * s + 512],
                                     start=False, stop=True,
                                     skip_group_check=True)
                nc.vector.tensor_scalar_add(
                    fr[:, c0 : c0 + CHUNK],
                    pq[:, :, :].rearrange("p s w -> p (s w)"), qkb)

                # vT projection: 2 rows per PSUM half-bank tile; bias is
                # added at evacuation (DVE), not via a PE matmul
                for s2 in range(4):
                    pv = pvpool.tile([128, 2, 130], F32, tag="pv")
                    for s3 in range(2):
                        srow = 2 * s2 + s3
                        nc.tensor.matmul(pv[:, s3, :], xav[:, srow, :], wva,
                                         start=(s3 == 0), stop=False,
                                         skip_group_check=True)
                        nc.tensor.matmul(pv[:, s3, :], xbv[:, srow, :], wvb,
                                         start=False, stop=(s3 == 1),
                                         skip_group_check=True)
                    nc.vector.tensor_tensor(
                        vT_sb[:, r0 + 2 * s2 : r0 + 2 * s2 + 2, :], pv,
                        vbias.to_broadcast((128, 2, 130)),
                        mybir.AluOpType.add)

                # col-major flat store slices: fc[:, :, h-slice] only needs
                # fr rows h-slice -> overlap the permute with projection
                if chi % 4 == 3:
                    hs = (chi // 4) * 32
                    frv = fr[:, :].rearrange("c (h w) -> c w h", w=W)
                    fcv = fc[:, :].rearrange("c (w h) -> c w h", h=H)
                    nc.vector.tensor_copy(fcv[:, :, hs : hs + 32],
                                          frv[:, :, hs : hs + 32])
                    # row-direction band-pack for the 8 groups just done
                    bandpack_rows(q_sb, k_sb, src_r, (chi // 4) * 8, 8,
                                  [nc.sync, nc.scalar])

            # column-direction band-pack (needs the full fc); spread over
            # three issuing engines
            bandpack_rows(qc_sb, kc_sb, src_c, 0, 32,
                          [nc.sync, nc.scalar, nc.gpsimd])

        # ---------------- Phase B2: vTc via PE transposes ----------------
        # vT[w, h, c] -> vTc[h, w, c]; per channel, batched 4 per bank.
        with tc.tile_pool(name="ptr", bufs=2, space="PSUM") as ptrpool:
            for cb in range(33):
                nch = min(4, 130 - cb * 4)
                ptr = ptrpool.tile([128, 4, 128], BF16, tag="ptr")
                for cj in range(nch):
                    cch = cb * 4 + cj
                    nc.tensor.matmul(ptr[:, cj, :], vT_sb[:, :, cch], identb,
                                     start=True, stop=True, is_transpose=True)
                dst = vTc_sb[:, :, cb * 4 : cb * 4 + nch]
                src = ptr[:, 0:nch, :].rearrange("p c w -> p w c")
                if cb % 2 == 0:
                    nc.vector.tensor_copy(dst, src)
                else:
                    nc.scalar.copy(dst, src)

        # ---------------- Phase C: attention (pipelined) ----------------
        with (
            tc.tile_pool(name="pe", bufs=2, space="PSUM") as pepool,
            tc.tile_pool(name="pt", bufs=4) as ptpool,
            tc.tile_pool(name="tt", bufs=3) as tpool,
            tc.tile_pool(name="au", bufs=2) as aupool,
            tc.tile_pool(name="rc", bufs=4) as rcpool,
            tc.tile_pool(name="io", bufs=3) as iopool,
        ):
            # pe is double-buffered (2 x 4 banks = all of PSUM): the po
            # value matmuls write INTO the same pe tile, overwriting each
            # energy slice right after its EXP consumed it, so the PE and
            # ACT never ping-pong on a shared single buffer.
            def energies(g):
                pe = pepool.tile([128, G, 512], F32, tag="pe")
                pTs = []
                for d in range(2):
                    qs = q_sb if d == 0 else qc_sb
                    ks = k_sb if d == 0 else kc_sb
                    for hh in range(2):
                        for j in range(G):
                            nc.tensor.matmul(
                                pe[:, j, 256 * d + 128 * hh :
                                   256 * d + 128 * hh + 128],
                                ks[32 * j : 32 * j + 8, hh, g, :],
                                qs[32 * j : 32 * j + 8, hh, g, :],
                                start=True, stop=True,
                                tile_position=(32 * j, 0),
                                skip_group_check=True,
                            )
                    pT = ptpool.tile([128, G, 256], BF16, tag="pt")
                    nc.scalar.activation(
                        pT, pe[:, :, 256 * d : 256 * d + 256],
                        mybir.ActivationFunctionType.Exp)
                    pTs.append(pT)
                return pe, pTs

            def attend(g, pe, pTs):
                for d in range(2):
                    vs = vT_sb if d == 0 else vTc_sb
                    for hh in range(2):
                        for j in range(G):
                            i = g * G + j
                            nc.tensor.matmul(
                                pe[:, j, 256 * d + 128 * hh :
                                   256 * d + 128 * hh + 65],
                                pTs[d][:, j, 128 * hh : 128 * hh + 128],
                                vs[:, i, 65 * hh : 65 * hh + 65],
                                start=True, stop=True,
                                skip_group_check=True,
                            )
                tils = []
                for d in range(2):
                    pov = pe[:, :, 256 * d : 256 * d + 256].rearrange(
                        "p j (hh c) -> p j hh c", hh=2)
                    til = tpool.tile([128, G, 2, CV], BF16, tag="t")
                    rec = rcpool.tile([128, G, 2, 1], F32, tag="rc")
                    nc.vector.reciprocal(rec, pov[:, :, :, 64:65])
                    nc.vector.tensor_tensor(
                        til, pov[:, :, :, 0:64],
                        rec.to_broadcast((128, G, 2, CV)),
                        mybir.AluOpType.mult,
                    )
                    tils.append(til)
                au = aupool.tile([128, G, 128], BF16, tag="au")
                nc.gpsimd.tensor_tensor(au, tils[0][:, :, :, :],
                                        tils[1][:, :, :, :],
                                        mybir.AluOpType.add)
                eng = nc.sync if g % 2 == 0 else nc.scalar
                xpg = iopool.tile([128, G * 128], BF16, tag="xpg")
                eng.dma_start(xpg, x_pix[:, g * 512 : (g + 1) * 512])
                res = iopool.tile([128, G * 128], BF16, tag="res")
                nc.vector.tensor_tensor(
                    res, au[:, :, :].rearrange("p g w -> p (g w)"),
                    xpg, mybir.AluOpType.add)
                eng.dma_start(out[:, g * 512 : (g + 1) * 512], res)

            prev = energies(0)
            for g in range(1, NG):
                cur = energies(g)
                attend(g - 1, *prev)
                prev = cur
            attend(NG - 1, *prev)

    return nc


def _prep_core_inputs(core, x, Wq, bq, Wk, bk, Wv, bv, gamma):
    b = core // 2
    p = core % 2
    g = float(np.asarray(gamma).reshape(-1)[0])
    qsl = slice(16 * p, 16 * p + 16)
    vsl = slice(128 * p, 128 * p + 128)

    import ml_dtypes
    bf = ml_dtypes.bfloat16

    # channel permutation: residual (output) channels first
    perm = np.concatenate([np.arange(128 * p, 128 * p + 128),
                           np.arange(128 * (1 - p), 128 * (1 - p) + 128)])

    wqk = np.zeros((C, 32), np.float32)
    wqk[:, 0:16] = Wq[qsl].T       # q head even(8) | q head odd(8)
    wqk[:, 16:32] = Wk[qsl].T
    wqk = wqk[perm].astype(bf)
    qkb = np.concatenate([bq[qsl], bk[qsl]]).reshape(32, 1).astype(np.float32)

    wv_eff = (g * Wv[vsl]).astype(np.float32)     # [128, 256]
    bv_eff = (g * bv[vsl]).astype(np.float32)
    wvt = np.zeros((C, 130), np.float32)
    wvt[:, 0:64] = wv_eff[0:64].T
    wvt[:, 65:129] = wv_eff[64:128].T
    wvt = wvt[perm].astype(bf)
    vbias = np.zeros((1, 130), np.float32)
    vbias[0, 0:64] = bv_eff[0:64]
    vbias[0, 64] = 1.0
    vbias[0, 65:129] = bv_eff[64:128]
    vbias[0, 129] = 1.0
    vbias_full = np.broadcast_to(vbias, (128, 130)).astype(bf)

    x2 = x[b].reshape(C, PIX)[perm]
    # pixel-major residual: [w][h][ch] so per-group loads are contiguous
    # pixel-major residual: x_pix[w, h*128+ch]
    xpix = np.ascontiguousarray(
        x[b, vsl].transpose(2, 1, 0)).reshape(W, H * 128)
    return {
        "x_in": np.ascontiguousarray(x2).astype(bf),
        "x_pix": xpix.astype(bf),
        "wqkT": wqk,
        "qk_bias": qkb,
        "wvT": wvt,
        "vbias_full": np.ascontiguousarray(vbias_full),
    }


_NC_CACHE = None


def _get_nc():
    global _NC_CACHE
    if _NC_CACHE is None:
        nc = build_program()
        nc.compile()
        _NC_CACHE = nc
    return _NC_CACHE


def kernel(x, Wq, bq, Wk, bk, Wv, bv, gamma, _trace=False, _trace_kwargs=None):
    from concourse.bass_utils import run_bass_kernel_spmd

    nc = _get_nc()
    in_maps = [
        _prep_core_inputs(core, x, Wq, bq, Wk, bk, Wv, bv, gamma)
        for core in range(NCORES)
    ]
    res = run_bass_kernel_spmd(
        nc, in_maps, list(range(NCORES)), trace=_trace,
        **(_trace_kwargs or {}),
    )
    outp = np.empty((B, C, H, W), np.float32)
    for core in range(NCORES):
        b, p = core // 2, core % 2
        o = res.results[core]["out"].astype(np.float32)
        # out[w, h*128+ch] -> [ch, h, w]
        outp[b, 128 * p : 128 * p + 128] = (
            o.reshape(W, H, 128).transpose(2, 1, 0)
        )
    if _trace:
        kernel.last_results = res
    return outp
